# revision 8
# baseline (speedup 1.0000x reference)
"""Trainium2 Bass kernel for structured-sparse matmul.

Computes: out[b,s,o] = sum_k x[b,s,sparse_idx[k]] * sparse_values[o,k]
  x: [4, 2048, 4096] f32, sparse_values: [4096, 1024] f32,
  sparse_idx: [1024] int64 (sorted, unique) -> out [4, 2048, 4096] f32

Strategy (8 NeuronCores, data-parallel over rows m = b*s), all-bf16:
  Host casts x / sparse_values to bf16 (end-to-end rel err ~3e-3, gate
  2e-2), pre-transposes each core's x slice to x^T m-halves
  [2, N_IN, 512] (pure layout prep), and lays sparse_idx out in the
  gpsimd wrapped-index format.
  Per core (M=1024 rows):
    Phase A runs entirely on the DMA path: gpsimd dma_gather pulls the
      1024 needed x^T rows straight from HBM into SBUF in the exact
      [k%128 part, k//128, m] GEMM layout (2 m-half calls; 2 MB instead
      of 8 MB of x traffic, ZERO PE cycles). Gather latency is dominated
      by gpsimd descriptor-gen (~8.4ns/desc + ~4us drain), so the first
      half lands ~12us in; the GEMM consumes half 0 across ALL o-slices
      (~57us of runway) before needing half 1.
    Phase B: GEMM out[m, o] = xg.T @ W^T[k, o] in bf16 (1 cycle/row).
      The PE stream is kept dense (fine-grained warmup bridging to the
      first gather, then 512 back-to-back [128x128x512] matmuls) so the
      HAM clock gate keeps the PE at 2.4 GHz - multi-us idles re-gate it
      to 1.2 GHz, which is what limited earlier variants.
  All 8 weight slices stay resident in SBUF (64 KB/partition in bf16).
  out written bf16 (8 MB/core) and upcast to f32 on the host.
"""

import sys

if "/opt/trn_rl_repo" not in sys.path:
    sys.path.insert(0, "/opt/trn_rl_repo")

import numpy as np
import ml_dtypes

BF16_NP = ml_dtypes.bfloat16

B, S, N_IN = 4, 2048, 4096
N_OUT, N_SPARSE = 4096, 1024
N_CORES = 8
M_TOT = B * S            # 8192
M = M_TOT // N_CORES     # 1024 rows per core
P = 128
NKT = N_SPARSE // P      # 8 k-tiles
N_MT = M // P            # 8 m-tiles per core
NH = 2                   # x^T gathered in m-halves
MH = M // NH             # 512
O_TILE = 512
NOS = N_OUT // O_TILE    # 8 o-slices
N_WARM = 220             # [128x64] warm matmuls bridging to first gather

_cache: dict = {}


def _build_nc():
    import concourse.mybir as mybir
    import concourse.tile as tile
    from concourse import bacc

    BF16 = mybir.dt.bfloat16
    F32 = mybir.dt.float32
    I16 = mybir.dt.int16

    # dynamic_dma_scratch_size: SWDGE descriptor-ring carveout. The default
    # 16384 gives a 1024-descriptor ring; a 1024-idx dma_gather never fits
    # and deadlocks the scheduler. 64KB -> 4096-descriptor ring.
    nc = bacc.Bacc(
        "TRN2",
        target_bir_lowering=False,
        debug=False,
        dynamic_dma_scratch_size=65536,
    )
    xt = nc.dram_tensor("xt", [NH, N_IN, MH], BF16, kind="ExternalInput")
    wt = nc.dram_tensor("wt", [NOS, P, NKT, O_TILE], BF16, kind="ExternalInput")
    idxs = nc.dram_tensor("idxs", [P, N_SPARSE // 16], I16, kind="ExternalInput")
    ident = nc.dram_tensor("ident", [P, P], BF16, kind="ExternalInput")
    out = nc.dram_tensor("out", [M, N_OUT], BF16, kind="ExternalOutput")

    with tile.TileContext(nc) as tc:
        with (
            tc.tile_pool(name="const", bufs=1) as const_pool,
            tc.tile_pool(name="xgpool", bufs=NH) as xg_pool,
            tc.tile_pool(name="wpool", bufs=NOS) as wt_pool,
            tc.tile_pool(name="opool", bufs=4) as o_pool,
            tc.tile_pool(name="ps_b", bufs=8, space="PSUM") as psb,
        ):
            idx_sb = const_pool.tile([P, N_SPARSE // 16], I16)
            nc.sync.dma_start(idx_sb[:], idxs[:])
            ident_sb = const_pool.tile([P, P], BF16)
            nc.sync.dma_start(ident_sb[:], ident[:])

            # Phase A: gather x^T rows from HBM, one call per m-half. Each
            # lands as xg_h[k%128, k//128, m_rel] - directly the GEMM's
            # stationary-operand layout.
            xg_tiles = []
            for h in range(NH):
                xg = xg_pool.tile([P, NKT, MH], BF16, tag="xg", name=f"xg{h}")
                nc.gpsimd.dma_gather(
                    xg[:],
                    xt[h],
                    idx_sb[:],
                    num_idxs=N_SPARSE,
                    num_idxs_reg=N_SPARSE,
                    elem_size=MH,
                )
                xg_tiles.append(xg)

            # All 8 wt slices resident; streamed on the scalar (ACT) ring.
            wt_tiles = []
            for s in range(NOS):
                t = wt_pool.tile([P, NKT, O_TILE], BF16, tag="wt", name=f"wt{s}")
                nc.scalar.dma_start(t[:], wt[s])
                wt_tiles.append(t)

            # PE warm-up: keep the PE busy (and the HAM clock gate open)
            # until the first gather lands (~12us). [128x64] bf16 matmuls are
            # ~27-107ns each, so the last one before the GEMM unblocks adds
            # little latency.
            for w in range(N_WARM):
                wps = psb.tile([P, O_TILE], F32, tag="psb", name=f"warm{w}")
                nc.tensor.matmul(
                    wps[:, :64], ident_sb[:], ident_sb[:, :64],
                    start=True, stop=True,
                )

            # Phase B: main GEMM, m-half outer so half 1 has ~57us of slack.
            for h in range(NH):
                xg = xg_tiles[h]
                for s in range(NOS):
                    wt_sb = wt_tiles[s]
                    for tt in range(N_MT // NH):
                        t = h * (N_MT // NH) + tt
                        ps = psb.tile([P, O_TILE], F32, tag="psb",
                                      name=f"psb{s}_{t}")
                        for kt in range(NKT):
                            nc.tensor.matmul(
                                ps[:],
                                xg[:, kt, tt * P:(tt + 1) * P],
                                wt_sb[:, kt, :],
                                start=(kt == 0),
                                stop=(kt == NKT - 1),
                            )
                        o_sb = o_pool.tile([P, O_TILE], BF16, tag="ob",
                                           name=f"ob{s}_{t}")
                        # Alternate eviction engine so neither DVE nor ACT
                        # gates PSUM recycling.
                        if t % 2 == 0:
                            nc.vector.tensor_copy(o_sb[:], ps[:])
                        else:
                            nc.scalar.copy(o_sb[:], ps[:])
                        nc.sync.dma_start(
                            out[t * P:(t + 1) * P,
                                s * O_TILE:(s + 1) * O_TILE],
                            o_sb[:],
                        )
    nc.compile()
    return nc


def _get_compiled():
    if "nc" not in _cache:
        _cache["nc"] = _build_nc()
    return _cache["nc"]


def _wrap_idx(idx: np.ndarray) -> np.ndarray:
    """gpsimd wrapped-index layout: idx i at [i%16, i//16], replicated
    across the 8 gpsimd core groups."""
    w = np.zeros((P, N_SPARSE // 16), dtype=np.int16)
    cols = idx.astype(np.int16).reshape(N_SPARSE // 16, 16)  # [col, part]
    for g in range(8):
        w[g * 16:(g + 1) * 16, :] = cols.T
    return w


def _run(inputs, trace=False, trace_kwargs=None):
    from concourse.bass_utils import run_bass_kernel_spmd

    x = np.asarray(inputs["x"], dtype=np.float32)
    sv = np.asarray(inputs["sparse_values"], dtype=np.float32)
    idx = np.asarray(inputs["sparse_idx"]).astype(np.int64)

    nc = _get_compiled()

    x2 = x.reshape(M_TOT, N_IN).astype(BF16_NP)
    # wt swizzled for contiguous per-partition DMA: [o-slice, k%128, k//128, o]
    wtv = np.ascontiguousarray(
        sv.T.reshape(NKT, P, NOS, O_TILE).transpose(2, 1, 0, 3).astype(BF16_NP)
    )
    idx_w = _wrap_idx(idx)
    ident = np.eye(P, dtype=BF16_NP)
    in_maps = [
        {
            # x^T m-halves: [h, n, m_rel] (layout-only prep)
            "xt": np.ascontiguousarray(
                x2[c * M:(c + 1) * M].T.reshape(N_IN, NH, MH).transpose(1, 0, 2)
            ),
            "wt": wtv,
            "idxs": idx_w,
            "ident": ident,
        }
        for c in range(N_CORES)
    ]
    res = run_bass_kernel_spmd(
        nc,
        in_maps,
        core_ids=list(range(N_CORES)),
        trace=trace,
        **(trace_kwargs or {}),
    )
    full = np.concatenate(
        [np.asarray(r["out"]).astype(np.float32) for r in res.results], axis=0
    )
    return full.reshape(B, S, N_OUT), res


def kernel(**inputs) -> np.ndarray:
    out, _ = _run(inputs)
    return out


# revision 10
# speedup vs baseline: 1.0492x; 1.0492x over previous
"""Trainium2 Bass kernel for structured-sparse matmul.

Computes: out[b,s,o] = sum_k x[b,s,sparse_idx[k]] * sparse_values[o,k]
  x: [4, 2048, 4096] f32, sparse_values: [4096, 1024] f32,
  sparse_idx: [1024] int64 (sorted, unique) -> out [4, 2048, 4096] f32

Strategy (8 NeuronCores, data-parallel over rows m = b*s), all-bf16
(end-to-end rel err ~3e-3, gate 2e-2). Per core (M=1024 rows, split in
4 m-quarters):
  Quarter 0 is prepared ON the PE while everything else warms up:
    regular row loads -> PE is_transpose matmuls (bf16, 1 cy/row,
    pass-through PSUM) -> one-hot G-matmul gather -> xg0[k%128, kt, m].
    This is useful work that keeps the PE busy (HAM clock gate stays
    open) during the ~22us fixed init of the gpsimd SWDGE path.
  Quarters 1-3 are gathered by gpsimd dma_gather straight from a
    host-pre-transposed x^T (pure layout prep): 1024 rows x 512B each
    land in the exact [k%128, kt, m] GEMM layout with zero PE cycles.
    Quarter 1 is k-split into 2x512-desc calls to beat its deadline
    (~40us); descriptor-gen is the latency driver (~10ns/desc + ~22us
    one-time init).
  GEMM (s-outer within each quarter): 512 back-to-back [128x128x512]
    bf16 matmuls at 2.4 GHz once ramped. All 8 weight slices resident
    (64 KB/partition bf16).
  out written bf16 (8 MB/core) and upcast to f32 on the host.
"""

import sys

if "/opt/trn_rl_repo" not in sys.path:
    sys.path.insert(0, "/opt/trn_rl_repo")

import numpy as np
import ml_dtypes

BF16_NP = ml_dtypes.bfloat16

B, S, N_IN = 4, 2048, 4096
N_OUT, N_SPARSE = 4096, 1024
N_CORES = 8
M_TOT = B * S            # 8192
M = M_TOT // N_CORES     # 1024 rows per core
P = 128
NKT = N_SPARSE // P      # 8 k-tiles
NNB = N_IN // P          # 32 n-blocks
N_MT = M // P            # 8 m-tiles per core
NQ = 4                   # m-quarters
MQ = M // NQ             # 256
O_TILE = 512
NOS = N_OUT // O_TILE    # 8 o-slices

_cache: dict = {}


def _build_gather_blocks(idx: np.ndarray):
    """Expand sparse_idx into one-hot selection blocks.

    For k-tile kt and n-block b, G[n, krel] = 1 iff idx[kt*128+krel] == b*128+n.
    Returns (g_all [NB,128,128] f32, blocks_per_kt: list of lists of (bi, b)).
    """
    mats = []
    blocks_per_kt = []
    for kt in range(NKT):
        ks = idx[kt * P:(kt + 1) * P]
        bs = sorted(set(int(k) // P for k in ks))
        entries = []
        for b in bs:
            mat = np.zeros((P, P), dtype=np.float32)
            for krel, k in enumerate(ks):
                if int(k) // P == b:
                    mat[int(k) % P, krel] = 1.0
            entries.append((len(mats), b))
            mats.append(mat)
        blocks_per_kt.append(entries)
    return np.stack(mats), blocks_per_kt


def _build_nc(blocks_per_kt, nb_total):
    import concourse.mybir as mybir
    import concourse.tile as tile
    from concourse import bacc

    BF16 = mybir.dt.bfloat16
    F32 = mybir.dt.float32
    I16 = mybir.dt.int16

    # dynamic_dma_scratch_size: SWDGE descriptor-ring carveout. The default
    # 16384 gives a 1024-descriptor ring; a 1024-idx dma_gather never fits
    # and deadlocks the scheduler. 64KB -> 4096-descriptor ring.
    nc = bacc.Bacc(
        "TRN2",
        target_bir_lowering=False,
        debug=False,
        dynamic_dma_scratch_size=65536,
    )
    xq0 = nc.dram_tensor("xq0", [MQ, N_IN], BF16, kind="ExternalInput")
    xt = nc.dram_tensor("xt", [NQ, N_IN, MQ], BF16, kind="ExternalInput")
    wt = nc.dram_tensor("wt", [NOS, P, NKT, O_TILE], BF16, kind="ExternalInput")
    g = nc.dram_tensor("g", [P, nb_total, P], BF16, kind="ExternalInput")
    idxs = nc.dram_tensor("idxs", [P, N_SPARSE // 16], I16, kind="ExternalInput")
    ident = nc.dram_tensor("ident", [P, P], BF16, kind="ExternalInput")
    out = nc.dram_tensor("out", [M, N_OUT], BF16, kind="ExternalOutput")

    with tile.TileContext(nc) as tc:
        with (
            tc.tile_pool(name="const", bufs=1) as const_pool,
            tc.tile_pool(name="xq0pool", bufs=1) as xq0_pool,
            tc.tile_pool(name="xtq0pool", bufs=1) as xtq0_pool,
            tc.tile_pool(name="gpool", bufs=1) as g_pool,
            tc.tile_pool(name="xgpool", bufs=NQ) as xg_pool,
            tc.tile_pool(name="wpool", bufs=NOS) as wt_pool,
            tc.tile_pool(name="opool", bufs=4) as o_pool,
            tc.tile_pool(name="ps_t", bufs=2, space="PSUM") as pst,
            tc.tile_pool(name="ps_g", bufs=2, space="PSUM") as psg,
            tc.tile_pool(name="ps_b", bufs=4, space="PSUM") as psb,
        ):
            idx_sb = const_pool.tile([P, N_SPARSE // 16], I16)
            nc.sync.dma_start(idx_sb[:], idxs[:])
            ident_sb = const_pool.tile([P, P], BF16)
            nc.sync.dma_start(ident_sb[:], ident[:])

            # Quarters 1-3 via gpsimd dma_gather (issued first; ~22us init
            # + desc-gen runs while the PE prepares quarter 0). Quarter 1 is
            # k-split so its halves land before the GEMM reaches it.
            xg_tiles = {0: xg_pool.tile([P, NKT, MQ], BF16, tag="xg",
                                        name="xg0")}
            for q in (1, 2, 3):
                xg_tiles[q] = xg_pool.tile([P, NKT, MQ], BF16, tag="xg",
                                           name=f"xg{q}")
            for q, k0, nk in ((1, 0, 4), (1, 4, 4), (2, 0, 8), (3, 0, 8)):
                nidx = nk * P
                nc.gpsimd.dma_gather(
                    xg_tiles[q][:, k0:k0 + nk, :],
                    xt[q],
                    idx_sb[:, (k0 * P) // 16:((k0 + nk) * P) // 16],
                    num_idxs=nidx,
                    num_idxs_reg=nidx,
                    elem_size=MQ,
                )

            # x quarter-0 rows stream on both HWDGE rings.
            xq0_sb = xq0_pool.tile([P, MQ // P, N_IN], BF16)
            # g + all 8 wt slices on the scalar ring (g first: gathers need
            # it ~5us in).
            g_sb = g_pool.tile([P, nb_total, P], BF16)
            nc.scalar.dma_start(g_sb[:], g[:])
            wt_tiles = []
            for s in range(NOS):
                t = wt_pool.tile([P, NKT, O_TILE], BF16, tag="wt",
                                 name=f"wt{s}")
                nc.scalar.dma_start(t[:], wt[s])
                wt_tiles.append(t)

            # Short PE warm-up (clock ramp) while the first x chunk lands.
            for w in range(3):
                wps = pst.tile([P, 4, P], BF16, tag="pst", name=f"warm{w}")
                for sl in range(4):
                    nc.tensor.transpose(wps[:, sl, :], ident_sb[:],
                                        ident_sb[:])

            # ---- Quarter 0 on the PE: transpose + gather, chasing loads ----
            xtq0 = xtq0_pool.tile([P, NNB, MQ], BF16)
            gathered = set()

            def emit_gather_q0(kt):
                entries = blocks_per_kt[kt]
                ps = psg.tile([P, MQ], F32, tag="psg", name=f"psg{kt}")
                for i, (bi, b) in enumerate(entries):
                    nc.tensor.matmul(
                        ps[:],
                        g_sb[:, bi, :],
                        xtq0[:, b, :],
                        start=(i == 0),
                        stop=(i == len(entries) - 1),
                    )
                if kt % 2 == 0:
                    nc.vector.tensor_copy(xg_tiles[0][:, kt, :], ps[:])
                else:
                    nc.scalar.copy(xg_tiles[0][:, kt, :], ps[:])

            NCH = 4           # n-chunks of 1024 cols
            CW = N_IN // NCH  # 1024
            NBC = CW // P     # 8 blocks per chunk
            for c in range(NCH):
                for j in range(MQ // P):
                    eng = nc.sync if (c + j) % 2 == 0 else nc.scalar
                    eng.dma_start(
                        xq0_sb[:, j, c * CW:(c + 1) * CW],
                        xq0[j * P:(j + 1) * P, c * CW:(c + 1) * CW],
                    )
                for j in range(MQ // P):
                    for half in range(2):
                        ps = pst.tile([P, 4, P], BF16, tag="pst")
                        for i in range(4):
                            nb = c * NBC + half * 4 + i
                            nc.tensor.transpose(
                                ps[:, i, :],
                                xq0_sb[:, j, nb * P:(nb + 1) * P],
                                ident_sb[:],
                            )
                        nb0 = c * NBC + half * 4
                        # strided dst (one 128-col m-slab per n-block)
                        nc.vector.tensor_copy(
                            xtq0[:, nb0:nb0 + 4, j * P:(j + 1) * P], ps[:]
                        )
                nb_done = (c + 1) * NBC
                for kt in range(NKT):
                    if kt not in gathered and all(
                        b < nb_done for _, b in blocks_per_kt[kt]
                    ):
                        emit_gather_q0(kt)
                        gathered.add(kt)
            for kt in range(NKT):
                if kt not in gathered:
                    emit_gather_q0(kt)

            # ---- Main GEMM: quarter-major so quarters 1-3 have slack ----
            for q in range(NQ):
                xg = xg_tiles[q]
                for s in range(NOS):
                    wt_sb = wt_tiles[s]
                    for t2 in range(N_MT // NQ):
                        t = q * (N_MT // NQ) + t2
                        ps = psb.tile([P, O_TILE], F32, tag="psb",
                                      name=f"psb{s}_{t}")
                        for kt in range(NKT):
                            nc.tensor.matmul(
                                ps[:],
                                xg[:, kt, t2 * P:(t2 + 1) * P],
                                wt_sb[:, kt, :],
                                start=(kt == 0),
                                stop=(kt == NKT - 1),
                            )
                        o_sb = o_pool.tile([P, O_TILE], BF16, tag="ob",
                                           name=f"ob{s}_{t}")
                        # Alternate eviction engine so neither DVE nor ACT
                        # gates PSUM recycling.
                        if t % 2 == 0:
                            nc.vector.tensor_copy(o_sb[:], ps[:])
                        else:
                            nc.scalar.copy(o_sb[:], ps[:])
                        nc.sync.dma_start(
                            out[t * P:(t + 1) * P,
                                s * O_TILE:(s + 1) * O_TILE],
                            o_sb[:],
                        )
    nc.compile()
    return nc


def _get_compiled(idx: np.ndarray):
    key = idx.tobytes()
    if key not in _cache:
        g_all, blocks_per_kt = _build_gather_blocks(idx)
        nc = _build_nc(blocks_per_kt, g_all.shape[0])
        _cache[key] = (nc, g_all)
    return _cache[key]


def _wrap_idx(idx: np.ndarray) -> np.ndarray:
    """gpsimd wrapped-index layout: within each 16-column group (one
    dma_gather call on 256 idx... generalized: idx i of a call starting
    at k0 sits at [i%16, k0//16 + i//16]), replicated across the 8 gpsimd
    core groups."""
    w = np.zeros((P, N_SPARSE // 16), dtype=np.int16)
    cols = idx.astype(np.int16).reshape(N_SPARSE // 16, 16)  # [col, part]
    for grp in range(8):
        w[grp * 16:(grp + 1) * 16, :] = cols.T
    return w


def _run(inputs, trace=False, trace_kwargs=None):
    from concourse.bass_utils import run_bass_kernel_spmd

    x = np.asarray(inputs["x"], dtype=np.float32)
    sv = np.asarray(inputs["sparse_values"], dtype=np.float32)
    idx = np.asarray(inputs["sparse_idx"]).astype(np.int64)

    nc, g_all = _get_compiled(idx)

    x2 = x.reshape(M_TOT, N_IN).astype(BF16_NP)
    # wt swizzled for contiguous per-partition DMA: [o-slice, k%128, k//128, o]
    wtv = np.ascontiguousarray(
        sv.T.reshape(NKT, P, NOS, O_TILE).transpose(2, 1, 0, 3).astype(BF16_NP)
    )
    g_swz = np.ascontiguousarray(g_all.transpose(1, 0, 2).astype(BF16_NP))
    idx_w = _wrap_idx(idx)
    ident = np.eye(P, dtype=BF16_NP)
    in_maps = []
    for c in range(N_CORES):
        xc = x2[c * M:(c + 1) * M]
        in_maps.append({
            "xq0": np.ascontiguousarray(xc[:MQ]),
            # x^T m-quarters: [q, n, m_rel] (layout-only prep)
            "xt": np.ascontiguousarray(
                xc.T.reshape(N_IN, NQ, MQ).transpose(1, 0, 2)
            ),
            "wt": wtv,
            "g": g_swz,
            "idxs": idx_w,
            "ident": ident,
        })
    res = run_bass_kernel_spmd(
        nc,
        in_maps,
        core_ids=list(range(N_CORES)),
        trace=trace,
        **(trace_kwargs or {}),
    )
    full = np.concatenate(
        [np.asarray(r["out"]).astype(np.float32) for r in res.results], axis=0
    )
    return full.reshape(B, S, N_OUT), res


def kernel(**inputs) -> np.ndarray:
    out, _ = _run(inputs)
    return out


# revision 12
# speedup vs baseline: 1.1975x; 1.1414x over previous
"""Trainium2 Bass kernel for structured-sparse matmul.

Computes: out[b,s,o] = sum_k x[b,s,sparse_idx[k]] * sparse_values[o,k]
  x: [4, 2048, 4096] f32, sparse_values: [4096, 1024] f32,
  sparse_idx: [1024] int64 (sorted, unique) -> out [4, 2048, 4096] f32

Strategy (8 NeuronCores, data-parallel over rows m = b*s), all-bf16
(end-to-end rel err ~3e-3, gate 2e-2). Per core (M=1024 rows, split in
4 m-quarters):
  Quarter 0 is prepared ON the PE while everything else warms up:
    regular row loads -> PE is_transpose matmuls (bf16, 1 cy/row,
    pass-through PSUM) -> one-hot G-matmul gather -> xg0[k%128, kt, m].
    This is useful work that keeps the PE busy (HAM clock gate stays
    open) during the ~22us fixed init of the gpsimd SWDGE path.
  Quarters 1-3 are gathered by gpsimd dma_gather straight from a
    host-pre-transposed x^T (pure layout prep): 1024 rows x 512B each
    land in the exact [k%128, kt, m] GEMM layout with zero PE cycles.
    Quarter 1 is k-split into 2x512-desc calls to beat its deadline
    (~40us); descriptor-gen is the latency driver (~10ns/desc + ~22us
    one-time init).
  GEMM (s-outer within each quarter): 512 back-to-back [128x128x512]
    bf16 matmuls at 2.4 GHz once ramped. All 8 weight slices resident
    (64 KB/partition bf16).
  out written bf16 (8 MB/core) and upcast to f32 on the host.
"""

import sys

if "/opt/trn_rl_repo" not in sys.path:
    sys.path.insert(0, "/opt/trn_rl_repo")

import numpy as np
import ml_dtypes

BF16_NP = ml_dtypes.bfloat16

B, S, N_IN = 4, 2048, 4096
N_OUT, N_SPARSE = 4096, 1024
N_CORES = 8
M_TOT = B * S            # 8192
M = M_TOT // N_CORES     # 1024 rows per core
P = 128
NKT = N_SPARSE // P      # 8 k-tiles
NNB = N_IN // P          # 32 n-blocks
N_MT = M // P            # 8 m-tiles per core
NQ = 4                   # m-quarters
MQ = M // NQ             # 256
O_TILE = 512
NOS = N_OUT // O_TILE    # 8 o-slices

_cache: dict = {}


def _build_gather_blocks(idx: np.ndarray):
    """Expand sparse_idx into one-hot selection blocks.

    For k-tile kt and n-block b, G[n, krel] = 1 iff idx[kt*128+krel] == b*128+n.
    Returns (g_all [NB,128,128] f32, blocks_per_kt: list of lists of (bi, b)).
    """
    mats = []
    blocks_per_kt = []
    for kt in range(NKT):
        ks = idx[kt * P:(kt + 1) * P]
        bs = sorted(set(int(k) // P for k in ks))
        entries = []
        for b in bs:
            mat = np.zeros((P, P), dtype=np.float32)
            for krel, k in enumerate(ks):
                if int(k) // P == b:
                    mat[int(k) % P, krel] = 1.0
            entries.append((len(mats), b))
            mats.append(mat)
        blocks_per_kt.append(entries)
    return np.stack(mats), blocks_per_kt


def _build_nc(blocks_per_kt, nb_total):
    import concourse.mybir as mybir
    import concourse.tile as tile
    from concourse import bacc

    BF16 = mybir.dt.bfloat16
    F32 = mybir.dt.float32
    I16 = mybir.dt.int16

    # dynamic_dma_scratch_size: SWDGE descriptor-ring carveout. The default
    # 16384 gives a 1024-descriptor ring; a 1024-idx dma_gather never fits
    # and deadlocks the scheduler. 64KB -> 4096-descriptor ring.
    nc = bacc.Bacc(
        "TRN2",
        target_bir_lowering=False,
        debug=False,
        dynamic_dma_scratch_size=65536,
    )
    xq0 = nc.dram_tensor("xq0", [MQ, N_IN], BF16, kind="ExternalInput")
    xt = nc.dram_tensor("xt", [NQ, N_IN, MQ], BF16, kind="ExternalInput")
    wt = nc.dram_tensor("wt", [NOS, P, NKT, O_TILE], BF16, kind="ExternalInput")
    g = nc.dram_tensor("g", [P, nb_total, P], BF16, kind="ExternalInput")
    idxs = nc.dram_tensor("idxs", [P, N_SPARSE // 16], I16, kind="ExternalInput")
    ident = nc.dram_tensor("ident", [P, P], BF16, kind="ExternalInput")
    out = nc.dram_tensor("out", [M, N_OUT], BF16, kind="ExternalOutput")

    with tile.TileContext(nc) as tc:
        with (
            tc.tile_pool(name="const", bufs=1) as const_pool,
            tc.tile_pool(name="xq0pool", bufs=1) as xq0_pool,
            tc.tile_pool(name="xtq0pool", bufs=1) as xtq0_pool,
            tc.tile_pool(name="gpool", bufs=1) as g_pool,
            tc.tile_pool(name="xgpool", bufs=NQ) as xg_pool,
            tc.tile_pool(name="wpool", bufs=NOS) as wt_pool,
            tc.tile_pool(name="opool", bufs=4) as o_pool,
            tc.tile_pool(name="ps_t", bufs=2, space="PSUM") as pst,
            tc.tile_pool(name="ps_g", bufs=2, space="PSUM") as psg,
            tc.tile_pool(name="ps_b", bufs=4, space="PSUM") as psb,
        ):
            idx_sb = const_pool.tile([P, N_SPARSE // 16], I16)
            nc.sync.dma_start(idx_sb[:], idxs[:])
            ident_sb = const_pool.tile([P, P], BF16)
            nc.sync.dma_start(ident_sb[:], ident[:])

            # Quarters 1-3 via gpsimd dma_gather (issued first; ~22us init
            # + desc-gen runs while the PE prepares quarter 0). Quarter 1 is
            # k-split so its halves land before the GEMM reaches it.
            xg_tiles = {0: xg_pool.tile([P, NKT, MQ], BF16, tag="xg",
                                        name="xg0")}
            for q in (1, 2, 3):
                xg_tiles[q] = xg_pool.tile([P, NKT, MQ], BF16, tag="xg",
                                           name=f"xg{q}")
            for q, k0, nk in ((1, 0, 4), (1, 4, 4), (2, 0, 8), (3, 0, 8)):
                nidx = nk * P
                nc.gpsimd.dma_gather(
                    xg_tiles[q][:, k0:k0 + nk, :],
                    xt[q],
                    idx_sb[:, (k0 * P) // 16:((k0 + nk) * P) // 16],
                    num_idxs=nidx,
                    num_idxs_reg=nidx,
                    elem_size=MQ,
                )

            # x quarter-0 rows stream FIRST on both HWDGE rings (the PE's
            # transposes chase them from ~1.5us); g and the 8 wt slices
            # queue behind them on the scalar ring in deadline order
            # (g ~5us, wt_s at ~12+3.6s us).
            NCH = 4           # n-chunks of 1024 cols
            CW = N_IN // NCH  # 1024
            NBC = CW // P     # 8 blocks per chunk
            xq0_sb = xq0_pool.tile([P, MQ // P, N_IN], BF16)
            for c in range(NCH):
                for j in range(MQ // P):
                    eng = nc.sync if (c + j) % 2 == 0 else nc.scalar
                    eng.dma_start(
                        xq0_sb[:, j, c * CW:(c + 1) * CW],
                        xq0[j * P:(j + 1) * P, c * CW:(c + 1) * CW],
                    )
            g_sb = g_pool.tile([P, nb_total, P], BF16)
            nc.scalar.dma_start(g_sb[:], g[:])
            wt_tiles = []
            for s in range(NOS):
                t = wt_pool.tile([P, NKT, O_TILE], BF16, tag="wt",
                                 name=f"wt{s}")
                nc.scalar.dma_start(t[:], wt[s])
                wt_tiles.append(t)

            # Short PE warm-up (clock ramp) while the first x chunk lands.
            for w in range(3):
                wps = pst.tile([P, 4, P], BF16, tag="pst", name=f"warm{w}")
                for sl in range(4):
                    nc.tensor.transpose(wps[:, sl, :], ident_sb[:],
                                        ident_sb[:])

            # ---- Quarter 0 on the PE: transpose + gather, chasing loads ----
            xtq0 = xtq0_pool.tile([P, NNB, MQ], BF16)
            gathered = set()

            def emit_gather_q0(kt):
                entries = blocks_per_kt[kt]
                ps = psg.tile([P, MQ], F32, tag="psg", name=f"psg{kt}")
                for i, (bi, b) in enumerate(entries):
                    nc.tensor.matmul(
                        ps[:],
                        g_sb[:, bi, :],
                        xtq0[:, b, :],
                        start=(i == 0),
                        stop=(i == len(entries) - 1),
                    )
                if kt % 2 == 0:
                    nc.vector.tensor_copy(xg_tiles[0][:, kt, :], ps[:])
                else:
                    nc.scalar.copy(xg_tiles[0][:, kt, :], ps[:])

            for c in range(NCH):
                for j in range(MQ // P):
                    for half in range(2):
                        ps = pst.tile([P, 4, P], BF16, tag="pst")
                        for i in range(4):
                            nb = c * NBC + half * 4 + i
                            nc.tensor.transpose(
                                ps[:, i, :],
                                xq0_sb[:, j, nb * P:(nb + 1) * P],
                                ident_sb[:],
                            )
                        nb0 = c * NBC + half * 4
                        # strided dst (one 128-col m-slab per n-block)
                        nc.vector.tensor_copy(
                            xtq0[:, nb0:nb0 + 4, j * P:(j + 1) * P], ps[:]
                        )
                nb_done = (c + 1) * NBC
                for kt in range(NKT):
                    if kt not in gathered and all(
                        b < nb_done for _, b in blocks_per_kt[kt]
                    ):
                        emit_gather_q0(kt)
                        gathered.add(kt)
            for kt in range(NKT):
                if kt not in gathered:
                    emit_gather_q0(kt)

            # ---- Main GEMM: quarter-major so quarters 1-3 have slack ----
            for q in range(NQ):
                xg = xg_tiles[q]
                for s in range(NOS):
                    wt_sb = wt_tiles[s]
                    for t2 in range(N_MT // NQ):
                        t = q * (N_MT // NQ) + t2
                        ps = psb.tile([P, O_TILE], F32, tag="psb",
                                      name=f"psb{s}_{t}")
                        for kt in range(NKT):
                            nc.tensor.matmul(
                                ps[:],
                                xg[:, kt, t2 * P:(t2 + 1) * P],
                                wt_sb[:, kt, :],
                                start=(kt == 0),
                                stop=(kt == NKT - 1),
                            )
                        o_sb = o_pool.tile([P, O_TILE], BF16, tag="ob",
                                           name=f"ob{s}_{t}")
                        # Alternate eviction engine so neither DVE nor ACT
                        # gates PSUM recycling.
                        if t % 2 == 0:
                            nc.vector.tensor_copy(o_sb[:], ps[:])
                        else:
                            nc.scalar.copy(o_sb[:], ps[:])
                        nc.sync.dma_start(
                            out[t * P:(t + 1) * P,
                                s * O_TILE:(s + 1) * O_TILE],
                            o_sb[:],
                        )
    nc.compile()
    return nc


def _get_compiled(idx: np.ndarray):
    key = idx.tobytes()
    if key not in _cache:
        g_all, blocks_per_kt = _build_gather_blocks(idx)
        nc = _build_nc(blocks_per_kt, g_all.shape[0])
        _cache[key] = (nc, g_all)
    return _cache[key]


def _wrap_idx(idx: np.ndarray) -> np.ndarray:
    """gpsimd wrapped-index layout: within each 16-column group (one
    dma_gather call on 256 idx... generalized: idx i of a call starting
    at k0 sits at [i%16, k0//16 + i//16]), replicated across the 8 gpsimd
    core groups."""
    w = np.zeros((P, N_SPARSE // 16), dtype=np.int16)
    cols = idx.astype(np.int16).reshape(N_SPARSE // 16, 16)  # [col, part]
    for grp in range(8):
        w[grp * 16:(grp + 1) * 16, :] = cols.T
    return w


def _run(inputs, trace=False, trace_kwargs=None):
    from concourse.bass_utils import run_bass_kernel_spmd

    x = np.asarray(inputs["x"], dtype=np.float32)
    sv = np.asarray(inputs["sparse_values"], dtype=np.float32)
    idx = np.asarray(inputs["sparse_idx"]).astype(np.int64)

    nc, g_all = _get_compiled(idx)

    x2 = x.reshape(M_TOT, N_IN).astype(BF16_NP)
    # wt swizzled for contiguous per-partition DMA: [o-slice, k%128, k//128, o]
    wtv = np.ascontiguousarray(
        sv.T.reshape(NKT, P, NOS, O_TILE).transpose(2, 1, 0, 3).astype(BF16_NP)
    )
    g_swz = np.ascontiguousarray(g_all.transpose(1, 0, 2).astype(BF16_NP))
    idx_w = _wrap_idx(idx)
    ident = np.eye(P, dtype=BF16_NP)
    in_maps = []
    for c in range(N_CORES):
        xc = x2[c * M:(c + 1) * M]
        in_maps.append({
            "xq0": np.ascontiguousarray(xc[:MQ]),
            # x^T m-quarters: [q, n, m_rel] (layout-only prep)
            "xt": np.ascontiguousarray(
                xc.T.reshape(N_IN, NQ, MQ).transpose(1, 0, 2)
            ),
            "wt": wtv,
            "g": g_swz,
            "idxs": idx_w,
            "ident": ident,
        })
    res = run_bass_kernel_spmd(
        nc,
        in_maps,
        core_ids=list(range(N_CORES)),
        trace=trace,
        **(trace_kwargs or {}),
    )
    full = np.concatenate(
        [np.asarray(r["out"]).astype(np.float32) for r in res.results], axis=0
    )
    return full.reshape(B, S, N_OUT), res


def kernel(**inputs) -> np.ndarray:
    out, _ = _run(inputs)
    return out
